# revision 12
# baseline (speedup 1.0000x reference)
"""BiLevelRoutingAttention (spiking, linear attention with window routing) on 8 TRN2 cores.

Sharding: 16 (t,b) pairs -> 2 per core, data-parallel. Host precomputes routing
(region sums -> top-k window indices) and ships x transposed as an f32r-grid hi
part plus a bf16 residual. The device does the qkv projection as 3 precision-
split products (hi@W_hi + hi@W_res in f32r, lo@W_bf16 in bf16), thresholds fused
into PSUM evacuation (DVE is_ge against a threshold tile for k/v, ACT sigmoid
saturation / DVE per-partition GE for q), per-window kv outer products (bf16),
top-k aggregation fused into indirect-DMA row gathers with compute_op=add,
block-diagonal unpack via identity matmuls, linear attention, and the output
projection (f32r, residual-split weights), producing bf16 0/1 output transposed;
host converts/transposes back.

Schedule: p0[kv+B] p0[q] p1[kv+B] p0[unpack+C+D] p1[q] p1[unpack+C+D] so the
serialized indirect gathers (~35us/pair on the gpsimd queue) hide behind matmul
phases of the other pair.
"""
import sys
sys.path.insert(0, '/opt/trn_rl_repo')

import numpy as np
import ml_dtypes

import concourse.bass as bass
import concourse.bacc as bacc
import concourse.mybir as mybir
from concourse.tile import TileContext
from concourse import bass_utils

F32 = mybir.dt.float32
F32R = mybir.dt.float32r
BF16 = mybir.dt.bfloat16
I32 = mybir.dt.int32
GE = mybir.AluOpType.is_ge
ADD = mybir.AluOpType.add
BYP = mybir.AluOpType.bypass
SIG = mybir.ActivationFunctionType.Sigmoid

T, B, L, C = 4, 4, 4096, 256
NW, TOPK, H, D = 8, 4, 4, 64
WIN = L // NW           # 512
NCORES = 8
NPAIR = 2               # (t,b) pairs per core
BIGS = 1.0e18           # sigmoid saturation scale

_EXEC_TIME_NS = None    # stashed for test harness


def _ensure_ntff_hook():
    """The agent image's antenv lacks axon_hooks; register the same hook
    trn_boot would have installed so trace=True can collect NTFF profiles."""
    import types
    try:
        import antenv.axon_hooks  # noqa: F401
        return True
    except ImportError:
        pass
    try:
        import antenv
        from trn_agent_boot.trn_boot import _ntff_profile_via_ctypes
        state = {"hook": _ntff_profile_via_ctypes('/opt/axon/libaxon_pjrt.so')}
        mod = types.ModuleType("antenv.axon_hooks")
        mod.get_axon_ntff_profile_hook = lambda: state["hook"]
        mod.set_axon_ntff_profile_hook = lambda h: state.__setitem__("hook", h)
        sys.modules["antenv.axon_hooks"] = mod
        antenv.axon_hooks = mod
        return True
    except Exception:
        return False


def _build_nc():
    nc = bacc.Bacc("TRN2", target_bir_lowering=False, debug=False,
                   num_devices=8)

    xhi = nc.dram_tensor("xhi", [NPAIR, C, L], F32, kind="ExternalInput")
    xlo = nc.dram_tensor("xlo", [NPAIR, C, L], BF16, kind="ExternalInput")
    wq = nc.dram_tensor("wq", [C, 768], F32, kind="ExternalInput")
    wqv = nc.dram_tensor("wqv", [C, 768], F32, kind="ExternalInput")
    wqb = nc.dram_tensor("wqb", [C, 768], BF16, kind="ExternalInput")
    wp = nc.dram_tensor("wp", [C, C], F32, kind="ExternalInput")
    wpv = nc.dram_tensor("wpv", [C, C], F32, kind="ExternalInput")
    thrkv = nc.dram_tensor("thrkv", [128, 512], F32, kind="ExternalInput")
    thrq = nc.dram_tensor("thrq", [128, 2], F32, kind="ExternalInput")
    sigbq = nc.dram_tensor("sigbq", [128, 2], F32, kind="ExternalInput")
    sigbp = nc.dram_tensor("sigbp", [128, 2], F32, kind="ExternalInput")
    idtop = nc.dram_tensor("idtop", [128, 128], BF16, kind="ExternalInput")
    idbot = nc.dram_tensor("idbot", [128, 128], BF16, kind="ExternalInput")
    idxrow = nc.dram_tensor("idxrow", [NPAIR, 128, NW * TOPK], I32, kind="ExternalInput")
    out = nc.dram_tensor("out", [NPAIR, C, L], BF16, kind="ExternalOutput")
    kvw_dram = [nc.dram_tensor(f"kvw_scratch{p}", [NW * 128, 128], BF16,
                               kind="Internal") for p in range(NPAIR)]

    with TileContext(nc) as tc:
        with (
            tc.tile_pool(name="const", bufs=1) as cpool,
            tc.tile_pool(name="xtp", bufs=1) as xtp,
            tc.tile_pool(name="big", bufs=1) as big,
            tc.tile_pool(name="qtp", bufs=1) as qtp,
            tc.tile_pool(name="kvwp", bufs=2) as kvwp,
            tc.tile_pool(name="gthp", bufs=1) as gthp,
            tc.tile_pool(name="agtp", bufs=2) as agtp,
            tc.tile_pool(name="kvgp", bufs=2) as kvgp,
            tc.tile_pool(name="bdp", bufs=4) as bdp,
            tc.tile_pool(name="otp", bufs=4) as otp,
            tc.tile_pool(name="finp", bufs=2) as finp,
            tc.tile_pool(name="psA", bufs=6, space="PSUM") as psA,
            tc.tile_pool(name="psB", bufs=2, space="PSUM") as psB,
        ):
            # ---- constants / weights (once) ----
            w_sb = [cpool.tile([128, 768], F32R, tag=f"wq{c}", name=f"wq{c}")
                    for c in range(2)]
            wv_sb = [cpool.tile([128, 768], F32R, tag=f"wv{c}", name=f"wv{c}")
                     for c in range(2)]
            wb_sb = [cpool.tile([128, 768], BF16, tag=f"wb{c}", name=f"wb{c}")
                     for c in range(2)]
            wp_sb = [cpool.tile([128, 256], F32R, tag=f"wp{c}", name=f"wp{c}")
                     for c in range(2)]
            wpv_sb = [cpool.tile([128, 256], F32R, tag=f"wpv{c}", name=f"wpv{c}")
                      for c in range(2)]
            # spread weight/const loads over several queues; kv-needed
            # columns (256:768) first so the first kv matmul starts early
            kvc = slice(256, 768)
            qc = slice(0, 256)
            for c in range(2):
                cs = slice(c * 128, (c + 1) * 128)
                nc.gpsimd.dma_start(w_sb[c][:, kvc], wq[cs, kvc].bitcast(F32R))
                nc.scalar.dma_start(wv_sb[c][:, kvc], wqv[cs, kvc].bitcast(F32R))
                nc.gpsimd.dma_start(wb_sb[c][:, kvc], wqb[cs, kvc])
            for c in range(2):
                cs = slice(c * 128, (c + 1) * 128)
                nc.gpsimd.dma_start(w_sb[c][:, qc], wq[cs, qc].bitcast(F32R))
                nc.scalar.dma_start(wv_sb[c][:, qc], wqv[cs, qc].bitcast(F32R))
                nc.gpsimd.dma_start(wb_sb[c][:, qc], wqb[cs, qc])
                nc.scalar.dma_start(wp_sb[c][:], wp[cs, :].bitcast(F32R))
                nc.scalar.dma_start(wpv_sb[c][:], wpv[cs, :].bitcast(F32R))
            thrkv_sb = cpool.tile([128, 512], F32, tag="thrkv", name="thrkv")
            nc.scalar.dma_start(thrkv_sb[:], thrkv[:])
            thrq_sb = cpool.tile([128, 2], F32, tag="thrq", name="thrq")
            nc.gpsimd.dma_start(thrq_sb[:], thrq[:])
            sigbq_sb = cpool.tile([128, 2], F32, tag="sigbq", name="sigbq")
            nc.gpsimd.dma_start(sigbq_sb[:], sigbq[:])
            sigbp_sb = cpool.tile([128, 2], F32, tag="sigbp", name="sigbp")
            nc.gpsimd.dma_start(sigbp_sb[:], sigbp[:])
            idt_sb = cpool.tile([128, 128], BF16, tag="idtop", name="idtop")
            nc.gpsimd.dma_start(idt_sb[:], idtop[:])
            idb_sb = cpool.tile([128, 128], BF16, tag="idbot", name="idbot")
            nc.gpsimd.dma_start(idb_sb[:], idbot[:])
            idx_sb = [cpool.tile([128, NW * TOPK], I32, tag=f"idx{p}", name=f"idx{p}")
                      for p in range(NPAIR)]

            # ---- x loads for both pairs, chunked for early start ----
            xhi_sb = [[xtp.tile([128, L], F32R, tag=f"xh{p}{c}", name=f"xh{p}{c}")
                       for c in range(2)] for p in range(NPAIR)]
            xlo_sb = [[xtp.tile([128, L], BF16, tag=f"xl{p}{c}", name=f"xl{p}{c}")
                       for c in range(2)] for p in range(NPAIR)]
            for p in range(NPAIR):
                # first chunk split finer so the first matmul starts sooner
                subs = ([slice(0, 256), slice(256, 1024)] if p == 0 else
                        [slice(0, 1024)])
                for q4 in range(4):
                    for qs in (subs if q4 == 0 else
                               [slice(q4 * 1024, (q4 + 1) * 1024)]):
                        for c in range(2):
                            cs = slice(c * 128, (c + 1) * 128)
                            nc.sync.dma_start(xhi_sb[p][c][:, qs],
                                              xhi[p, cs, qs].bitcast(F32R))
                            nc.sync.dma_start(xlo_sb[p][c][:, qs], xlo[p, cs, qs])
                nc.sync.dma_start(idx_sb[p][:], idxrow[p, :, :])

            kv_sb = big.tile([128, 32 * 512], BF16, tag="kv", name="kv")
            qt_sb = [[qtp.tile([128, L], BF16, tag=f"qt{p}{dq}", name=f"qt{p}{dq}")
                      for dq in range(2)] for p in range(NPAIR)]
            gath_sb = gthp.tile([128, 4 * TOPK * 128], BF16, tag="gath",
                                name="gath")
            kvg_sb = [kvgp.tile([128, NW * 128], BF16, tag="kvg", name=f"kvg{p}")
                      for p in range(NPAIR)]

            def kv_proj_and_B(p):
                """k/v projection + per-window kvw, interleaved; kvw -> DRAM."""
                def emit_B(duo):
                    kvwf = psB.tile([128, 512], F32, tag="kvwf", name="kvwf")
                    for jl in range(2):
                        j = duo * 2 + jl
                        for hp in range(2):
                            blk = (2 * jl + hp) * 128
                            for cc in range(4):
                                col = (4 * j + cc) * 512
                                nc.tensor.matmul(
                                    kvwf[:, blk:blk + 128],
                                    kv_sb[:, col + hp * 128: col + hp * 128 + 128],
                                    kv_sb[:, col + 256 + hp * 128: col + 256 + hp * 128 + 128],
                                    start=(jl == 0 and hp == 0 and cc == 0),
                                    stop=(jl == 1 and hp == 1 and cc == 3),
                                    skip_group_check=True)
                    # extract per-head diagonal blocks -> packed kvw tile
                    kvw_t = kvwp.tile([128, 256], BF16, tag="kvwt", name="kvwt")
                    for s in range(2):
                        srows = slice(s * 64, (s + 1) * 64)
                        srcap = kvwf[srows, :].rearrange(
                            "q (b e) -> q b e", e=128)[:, :, s * 64:s * 64 + 64]
                        dstap = kvw_t[srows, :].rearrange(
                            "q (b e) -> q b e", e=64)
                        if s == 0:
                            nc.vector.tensor_copy(dstap, srcap)
                        else:
                            nc.scalar.copy(dstap, srcap)
                    for jl in range(2):
                        j = duo * 2 + jl
                        nc.sync.dma_start(
                            kvw_dram[p][j * 128:(j + 1) * 128, :],
                            kvw_t[:, jl * 128:(jl + 1) * 128])

                for m in range(32):
                    ps = psA.tile([128, 512], F32, tag="psA", name="psA")
                    msl = slice(m * 128, (m + 1) * 128)
                    nc.tensor.matmul(ps[:], xhi_sb[p][0][:, msl],
                                     w_sb[0][:, 256:768], start=True, stop=False,
                                     skip_group_check=True)
                    nc.tensor.matmul(ps[:], xhi_sb[p][0][:, msl],
                                     wv_sb[0][:, 256:768], start=False, stop=False,
                                     skip_group_check=True)
                    nc.tensor.matmul(ps[:], xhi_sb[p][1][:, msl],
                                     w_sb[1][:, 256:768], start=False, stop=False,
                                     skip_group_check=True)
                    nc.tensor.matmul(ps[:], xhi_sb[p][1][:, msl],
                                     wv_sb[1][:, 256:768], start=False, stop=False,
                                     skip_group_check=True)
                    nc.tensor.matmul(ps[:], xlo_sb[p][0][:, msl],
                                     wb_sb[0][:, 256:768], start=False, stop=False,
                                     skip_group_check=True)
                    nc.tensor.matmul(ps[:], xlo_sb[p][1][:, msl],
                                     wb_sb[1][:, 256:768], start=False, stop=True,
                                     skip_group_check=True)
                    nc.vector.tensor_tensor(kv_sb[:, m * 512:(m + 1) * 512],
                                            ps[:], thrkv_sb[:], GE)
                    if m % 8 == 0 and m > 0:
                        emit_B(m // 8 - 1)
                emit_B(3)

            def q_proj(p):
                nevac = 0
                for g in range(8):
                    for dq in range(2):
                        ps = psA.tile([128, 512], F32, tag="psA", name="psA")
                        gsl = slice(g * 512, (g + 1) * 512)
                        dsl = slice(dq * 128, (dq + 1) * 128)
                        nc.tensor.matmul(ps[:], w_sb[0][:, dsl], xhi_sb[p][0][:, gsl],
                                         start=True, stop=False, skip_group_check=True)
                        nc.tensor.matmul(ps[:], wv_sb[0][:, dsl], xhi_sb[p][0][:, gsl],
                                         start=False, stop=False, skip_group_check=True)
                        nc.tensor.matmul(ps[:], w_sb[1][:, dsl], xhi_sb[p][1][:, gsl],
                                         start=False, stop=False, skip_group_check=True)
                        nc.tensor.matmul(ps[:], wv_sb[1][:, dsl], xhi_sb[p][1][:, gsl],
                                         start=False, stop=False, skip_group_check=True)
                        nc.tensor.matmul(ps[:], wb_sb[0][:, dsl], xlo_sb[p][0][:, gsl],
                                         start=False, stop=False, skip_group_check=True)
                        nc.tensor.matmul(ps[:], wb_sb[1][:, dsl], xlo_sb[p][1][:, gsl],
                                         start=False, stop=True, skip_group_check=True)
                        dst = qt_sb[p][dq][:, gsl]
                        if nevac % 2 == 0:
                            nc.scalar.activation(dst, ps[:], SIG,
                                                 bias=sigbq_sb[:, dq:dq + 1], scale=BIGS)
                        else:
                            nc.vector.tensor_scalar(dst, ps[:], thrq_sb[:, dq:dq + 1],
                                                    None, GE)
                        nevac += 1

            def gathers(p):
                # plain gathers (RMW-add gathers measured ~2x slower), then
                # 3 adds per window on gpsimd, which is idle right after its
                # own gather stream
                for half in range(2):
                    for n in range(half * 4, half * 4 + 4):
                        for i in range(TOPK):
                            m = n * TOPK + i
                            sl = ((n % 4) * TOPK + i) * 128
                            nc.gpsimd.indirect_dma_start(
                                out=gath_sb[:, sl:sl + 128],
                                out_offset=None,
                                in_=kvw_dram[p][:],
                                in_offset=bass.IndirectOffsetOnAxis(
                                    ap=idx_sb[p][:, m:m + 1], axis=0),
                            )
                    for n in range(half * 4, half * 4 + 4):
                        base = (n % 4) * TOPK * 128
                        g01 = gath_sb[:, base:base + 256]
                        g23 = gath_sb[:, base + 256:base + 512]
                        t2 = agtp.tile([128, 256], F32, tag="agt", name="agt2")
                        nc.gpsimd.tensor_tensor(t2[:], g01, g23, ADD)
                        nc.gpsimd.tensor_tensor(kvg_sb[p][:, n * 128:(n + 1) * 128],
                                                t2[:, 0:128], t2[:, 128:256], ADD)

            def unpack_cd(p):
                # unpack aggregated packed tiles -> block-diag kvg_bd (2 windows
                # per PSUM tile), then per-window C (out = kvg^T-as-lhsT @ q^T)
                # and D (proj + threshold), staggered to hide evac latency.
                bd = []
                for base in range(0, NW, 2):
                    psu = psA.tile([128, 512], F32, tag="psA", name="psU")
                    for nl in range(2):
                        n = base + nl
                        bap = psu[:, nl * 256:(nl + 1) * 256].rearrange(
                            "q (hp e) -> q hp e", hp=2)
                        top = bap[:, :, 0:64]
                        bot = bap[:, :, 64:128]
                        rhs = kvg_sb[p][:, n * 128:(n + 1) * 128]
                        nc.tensor.matmul(top, idt_sb[:], rhs,
                                         start=(nl == 0), stop=False,
                                         skip_group_check=True)
                        nc.tensor.matmul(bot, idb_sb[:], rhs,
                                         start=False, stop=(nl == 1),
                                         skip_group_check=True)
                    t = bdp.tile([128, 512], BF16, tag="bd", name="bd")
                    if base % 4 == 0:
                        nc.vector.tensor_copy(t[:], psu[:])
                    else:
                        nc.scalar.copy(t[:], psu[:])
                    bd.append(t)

                def emit_C(n):
                    ots = []
                    for hp in range(2):
                        ps = psA.tile([128, 512], F32, tag="psA", name="psCt")
                        nc.tensor.matmul(
                            ps[:],
                            bd[n // 2][:, (n % 2) * 256 + hp * 128:
                                       (n % 2) * 256 + hp * 128 + 128],
                            qt_sb[p][hp][:, n * 512:(n + 1) * 512],
                            start=True, stop=True)
                        ot = otp.tile([128, 512], F32R, tag="ot", name="ot")
                        if hp == 0:
                            nc.vector.tensor_copy(ot[:], ps[:])
                        else:
                            nc.scalar.copy(ot[:], ps[:])
                        ots.append(ot)
                    return ots

                def emit_D(n, ots):
                    for ct in range(2):
                        ps = psA.tile([128, 512], F32, tag="psA", name="psD")
                        csl = slice(ct * 128, (ct + 1) * 128)
                        nc.tensor.matmul(ps[:], wp_sb[0][:, csl], ots[0][:],
                                         start=True, stop=False)
                        nc.tensor.matmul(ps[:], wpv_sb[0][:, csl], ots[0][:],
                                         start=False, stop=False)
                        nc.tensor.matmul(ps[:], wp_sb[1][:, csl], ots[1][:],
                                         start=False, stop=False)
                        nc.tensor.matmul(ps[:], wpv_sb[1][:, csl], ots[1][:],
                                         start=False, stop=True)
                        fin = finp.tile([128, 512], BF16, tag="fin", name="fin")
                        nc.scalar.activation(fin[:], ps[:], SIG,
                                             bias=sigbp_sb[:, ct:ct + 1], scale=BIGS)
                        eng = nc.sync if (n + ct) % 2 == 0 else nc.gpsimd
                        eng.dma_start(
                            out[p, ct * 128:(ct + 1) * 128, n * 512:(n + 1) * 512],
                            fin[:])

                prev = emit_C(0)
                for n in range(1, NW):
                    cur = emit_C(n)
                    emit_D(n - 1, prev)
                    prev = cur
                emit_D(NW - 1, prev)

            kv_proj_and_B(0)
            gathers(0)
            kv_proj_and_B(1)
            gathers(1)
            q_proj(0)
            q_proj(1)
            unpack_cd(0)
            unpack_cd(1)

    nc.compile()
    return nc


_NC = None


def _f32r_rne(a):
    """Round fp32 to the f32r grid (12-bit significand)."""
    u = np.ascontiguousarray(a, dtype=np.float32).view(np.uint32)
    u = (u + np.uint32(1 << 11)) & np.uint32(0xFFFFF000)
    return u.view(np.float32)


def kernel(x, W_qkv, b_qkv, W_proj, b_proj):
    global _NC, _EXEC_TIME_NS
    x = np.asarray(x, dtype=np.float32)
    W_qkv = np.asarray(W_qkv, dtype=np.float32)
    b_qkv = np.asarray(b_qkv, dtype=np.float32)
    W_proj = np.asarray(W_proj, dtype=np.float32)
    b_proj = np.asarray(b_proj, dtype=np.float32)

    # ---- host routing: region sums -> attn -> top-k window indices ----
    region = x.sum(axis=0).reshape(B, NW, WIN, C).sum(axis=2)        # [B,NW,C]
    attn_r = np.einsum('bnc,bmc->bnm', region, region)
    idx = np.argsort(-attn_r, axis=-1, kind='stable')[:, :, :TOPK]   # [B,NW,TOPK]

    wq_u = _f32r_rne(W_qkv)
    wp_u = _f32r_rne(W_proj)
    common = {
        "wq": wq_u,
        "wqv": np.ascontiguousarray(W_qkv - wq_u),
        "wqb": W_qkv.astype(ml_dtypes.bfloat16),
        "wp": wp_u,
        "wpv": np.ascontiguousarray(W_proj - wp_u),
        "thrkv": np.broadcast_to(2.0 - b_qkv[None, 256:768], (128, 512)).astype(np.float32).copy(),
        "thrq": np.ascontiguousarray((2.0 - b_qkv[0:256]).reshape(2, 128).T),
        "sigbq": np.ascontiguousarray(-BIGS * (2.0 - b_qkv[0:256]).reshape(2, 128).T).astype(np.float32),
        "sigbp": np.ascontiguousarray(-BIGS * (2.0 - b_proj).reshape(2, 128).T).astype(np.float32),
        "idtop": np.diag(np.r_[np.ones(64), np.zeros(64)]).astype(ml_dtypes.bfloat16),
        "idbot": np.diag(np.r_[np.zeros(64), np.ones(64)]).astype(ml_dtypes.bfloat16),
    }

    in_maps = []
    pairs = [(t, b) for t in range(T) for b in range(B)]
    for core in range(NCORES):
        mine = pairs[core * NPAIR:(core + 1) * NPAIR]
        xt_full = np.stack([np.ascontiguousarray(x[t, b].T) for (t, b) in mine])
        xh = _f32r_rne(xt_full)
        xl = (xt_full - xh).astype(ml_dtypes.bfloat16)
        rows = []
        for k, (t, b) in enumerate(mine):
            r = np.empty((128, NW * TOPK), dtype=np.int32)
            for n in range(NW):
                for i in range(TOPK):
                    r[:, n * TOPK + i] = idx[b, n, i] * 128 + np.arange(128)
            rows.append(r)
        m = dict(common)
        m["xhi"] = xh
        m["xlo"] = xl
        m["idxrow"] = np.stack(rows)
        in_maps.append(m)

    if _NC is None:
        _NC = _build_nc()

    traceable = _ensure_ntff_hook()
    try:
        res = bass_utils.run_bass_kernel_spmd(_NC, in_maps,
                                              core_ids=list(range(NCORES)),
                                              trace=traceable)
    except Exception:
        if not traceable:
            raise
        res = bass_utils.run_bass_kernel_spmd(_NC, in_maps,
                                              core_ids=list(range(NCORES)),
                                              trace=False)
    _EXEC_TIME_NS = res.exec_time_ns

    full = np.empty((T, B, L, C), dtype=np.float32)
    for core in range(NCORES):
        mine = pairs[core * NPAIR:(core + 1) * NPAIR]
        o = res.results[core]["out"]                                  # [NPAIR, C, L] bf16
        for k, (t, b) in enumerate(mine):
            full[t, b] = o[k].astype(np.float32).T
    return full
